# revision 17
# baseline (speedup 1.0000x reference)
"""Bidirectional ELU-RNN encoder kernel for Trainium2 (8 NeuronCores).

Sharding: data-parallel over (batch x direction): cores 0-3 run the
left-to-right direction on batch quarters 0-3, cores 4-7 the
right-to-left direction (host reverses time for those cores so every
core runs an identical forward recurrence).

Per core (B=16 sequences, S=512 steps, EMB=512, HID=1024):
  phase 1: indirect-DMA gather of E rows + PE transpose -> embT tiles
  phase 2: X = emb @ We.T + b as a big GEMM (f32r), X -> DRAM
  phase 3: recurrence h_t = elu(X_t + h_{t-1} @ Wh.T)
     - h kept transposed (hT [128, 8kt x 16b]) so it feeds matmul lhsT
     - X_t seeded into PSUM by engine copy, matmuls accumulate on top
     - pre-activation transposed back by PE transposes; ELU computed as
       relu(a) - relu(1 - exp(a)) on full-width [128, *] tiles
"""

import os
import sys
import tempfile

sys.path.insert(0, "/opt/trn_rl_repo")

import numpy as np

from concourse import bass, bacc, mybir
from concourse import hw_specs
from concourse.tile import TileContext
from concourse.bass_utils import run_bass_kernel_spmd

# Prefer the activation-function table that holds BOTH Exp and Ln so the
# per-step Exp/Ln alternation doesn't reload tables (1.28us each).
_orig_get_act_tables = hw_specs.get_activation_tables


def _act_tables_ln_exp_first(module_arch):
    tabs = _orig_get_act_tables(module_arch)
    if "natural_log_exp_and_others" not in tabs:
        return tabs
    exp = mybir.ActivationFunctionType.Exp
    ln = mybir.ActivationFunctionType.Ln
    out = {}
    for name, fns in tabs.items():
        if name != "natural_log_exp_and_others":
            fns = fns - {exp, ln}
        out[name] = fns
    return out


hw_specs.get_activation_tables = _act_tables_ln_exp_first
bacc.get_activation_tables = _act_tables_ln_exp_first

f32 = mybir.dt.float32
f32r = mybir.dt.float32r
i32 = mybir.dt.int32

VOCAB = 32000
EMB = 512
HID = 1024
BATCH = 64
SEQ = 512

BL = 16          # sequences per core
NTOK = SEQ * BL  # 8192 tokens per core
NMT = NTOK // 128  # 64 token m-tiles
SBLK = 8         # steps per X-block DMA
ACT = mybir.ActivationFunctionType


def build_nc():
    nc = bacc.Bacc()

    E = nc.dram_tensor("E", [VOCAB, EMB], f32, kind="ExternalInput")
    idx = nc.dram_tensor("idx", [128, NMT], i32, kind="ExternalInput")
    WhT = nc.dram_tensor("WhT", [128, 8 * HID], f32r, kind="ExternalInput")
    WeT = nc.dram_tensor("WeT", [128, 4 * HID], f32r, kind="ExternalInput")
    bias = nc.dram_tensor("bias", [1, HID], f32r, kind="ExternalInput")
    ones = nc.dram_tensor("ones", [1, 128], f32r, kind="ExternalInput")
    i16 = nc.dram_tensor("i16", [BL, BL], f32r, kind="ExternalInput")
    hin = nc.dram_tensor("hinit", [128, 128], f32r, kind="ExternalInput")
    ident = nc.dram_tensor("ident", [128, 128], f32, kind="ExternalInput")
    hsT = nc.dram_tensor("hsT", [SEQ, 128, 128], f32, kind="ExternalOutput")

    with TileContext(nc) as tc:
        with tc.tile_pool(name="const", bufs=1) as cp, \
             tc.tile_pool(name="gather", bufs=3) as gp, \
             tc.tile_pool(name="xio", bufs=2) as xp, \
             tc.tile_pool(name="work", bufs=4) as wp, \
             tc.tile_pool(name="hbuf", bufs=2) as hp, \
             tc.tile_pool(name="dram", bufs=1, space="DRAM") as dp, \
             tc.tile_pool(name="psbig", bufs=2, space="PSUM") as psb, \
             tc.tile_pool(name="pssm", bufs=2, space="PSUM") as pss:

            # ---- constants / weights ----
            whT = cp.tile([128, 8 * HID], f32r)
            weT = cp.tile([128, 4 * HID], f32r)
            bia = cp.tile([1, HID], f32r)
            one = cp.tile([1, 128], f32r)
            idn = cp.tile([128, 128], f32)
            ixt = cp.tile([128, NMT], i32)
            id16 = cp.tile([BL, BL], f32r)
            nc.sync.dma_start(id16[:], i16[:])
            hinit = cp.tile([128, 128], f32r)
            nc.sync.dma_start(hinit[:], hin[:])
            nc.sync.dma_start(whT[:], WhT[:])
            nc.sync.dma_start(weT[:], WeT[:])
            nc.sync.dma_start(bia[:], bias[:])
            nc.sync.dma_start(one[:], ones[:])
            nc.sync.dma_start(idn[:], ident[:])
            nc.sync.dma_start(ixt[:], idx[:])

            X = dp.tile([NTOK, HID], f32r)

            # ---- phase 1+2: gather + input projection ----
            for m in range(NMT):
                g = gp.tile([128, EMB], f32, tag="g")
                nc.gpsimd.indirect_dma_start(
                    out=g[:], out_offset=None, in_=E[:],
                    in_offset=bass.IndirectOffsetOnAxis(
                        ap=ixt[:, m:m + 1], axis=0))
                embT = gp.tile([128, EMB], f32r, tag="embT")
                for q in range(4):
                    tp = pss.tile([128, 128], f32, tag="trE")
                    nc.tensor.transpose(
                        tp[:], g[:, 128 * q:128 * (q + 1)], idn[:])
                    nc.vector.tensor_copy(
                        embT[:, 128 * q:128 * (q + 1)], tp[:])
                xps = psb.tile([128, HID], f32, tag="big")
                for c in range(2):
                    oc = 512 * c
                    for k in range(4):
                        nc.tensor.matmul(
                            xps[:, oc:oc + 512],
                            lhsT=embT[:, 128 * k:128 * (k + 1)],
                            rhs=weT[:, HID * k + oc:HID * k + oc + 512],
                            start=(k == 0), stop=False)
                    nc.tensor.matmul(
                        xps[:, oc:oc + 512], lhsT=one[:1, :],
                        rhs=bia[:1, oc:oc + 512], start=False, stop=True)
                xsb = gp.tile([128, HID], f32r, tag="xsb")
                nc.vector.tensor_copy(xsb[:], xps[:])
                nc.sync.dma_start(X[128 * m:128 * (m + 1), :], xsb[:])

            # ---- phase 3: recurrence (software-pipelined) ----
            xv = X[:].rearrange("(s b) j -> s b j", b=BL)
            xbs = {}

            def load_xblk(blk):
                xb = xp.tile([BL, SBLK * HID], f32r, tag="xb")
                nc.sync.dma_start(
                    xb[:].rearrange("p (s j) -> p s j", s=SBLK),
                    xv[SBLK * blk:SBLK * (blk + 1), :, :].rearrange(
                        "s b j -> b s j"))
                xbs[blk] = xb

            def seed(t):
                # X_t -> PSUM via a K=16 identity matmul on PE
                blk, s = divmod(t, SBLK)
                if s == 0:
                    load_xblk(blk)
                a = psb.tile([128, HID], f32, tag="big")
                av = a[0:BL, :]
                xb = xbs[blk]
                for c in range(2):
                    oc = 512 * c
                    nc.tensor.matmul(
                        av[:, oc:oc + 512], lhsT=id16[:],
                        rhs=xb[:, HID * s + oc:HID * s + oc + 512],
                        start=True, stop=False, skip_group_check=True)
                return a

            def elu_exp(av, c):
                # Shifted ELU: g = h + 1 = min(exp(a), relu(a) + 1), with
                # relu(a) recovered as max(ln(exp(a)) + 1, 1) so the psum
                # chunk is read exactly once (by ACT exp). The +1 shift is
                # absorbed by bias b' = b - Wh@1 on the host; outputs store
                # g and the host subtracts 1.
                oc = 512 * c
                ec = wp.tile([BL, 512], f32, tag="ec")
                nc.scalar.activation(ec[:], av[:, oc:oc + 512], ACT.Exp)
                return ec

            def elu_tail(ec, c, h_new):
                trE = pss.tile([128, 64], f32, tag="trE")
                for q in range(4):
                    nc.tensor.transpose(
                        trE[:, 16 * q:16 * (q + 1)],
                        ec[:, 128 * q:128 * (q + 1)], idn[0:BL, 0:BL])
                ln = wp.tile([128, 64], f32, tag="ln")
                nc.scalar.activation(ln[:], trE[:], ACT.Ln)
                r1 = wp.tile([128, 64], f32, tag="r1")
                nc.vector.tensor_scalar(
                    out=r1[:], in0=ln[:], scalar1=1.0, scalar2=1.0,
                    op0=mybir.AluOpType.add, op1=mybir.AluOpType.max)
                nc.vector.tensor_tensor(
                    out=h_new[:, 64 * c:64 * (c + 1)], in0=trE[:], in1=r1[:],
                    op=mybir.AluOpType.min)

            def elu_chunk(av, c, h_new):
                elu_tail(elu_exp(av, c), c, h_new)

            def mm_group(av, h_prev, c, klo):
                oc = 512 * c
                for kt in range(klo, klo + 4):
                    nc.tensor.matmul(
                        av[:, oc:oc + 512],
                        lhsT=h_prev[:, 16 * kt:16 * (kt + 1)],
                        rhs=whT[:, HID * kt + oc:HID * kt + oc + 512],
                        start=False, stop=(kt == 7), skip_group_check=True)

            # Software pipeline: chunk-1's post-exp tail (transpose, ln,
            # min) is deferred into the next iteration, emitted after that
            # iteration's first ready mm group, so the in-order PE stream
            # never idles waiting on the exp of the freshest chunk.
            h_prev = hinit
            a_cur = seed(0)
            pend = None  # (ec, h_tile, t) for the deferred c1 tail
            for t in range(SEQ):
                av = a_cur[0:BL, :]
                h_new = hp.tile([128, 128], f32r, tag="hT")
                mm_group(av, h_prev, 0, 0)
                if pend is not None:
                    pec, ph, pt = pend
                    elu_tail(pec, 1, ph)
                    nc.sync.dma_start(hsT[pt], ph[:].bitcast(f32))
                mm_group(av, h_prev, 0, 4)
                mm_group(av, h_prev, 1, 0)
                elu_chunk(av, 0, h_new)
                mm_group(av, h_prev, 1, 4)
                if t < SEQ - 1:
                    a_cur = seed(t + 1)
                ec1 = elu_exp(av, 1)
                pend = (ec1, h_new, t)
                h_prev = h_new
            pec, ph, pt = pend
            elu_tail(pec, 1, ph)
            nc.sync.dma_start(hsT[pt], ph[:].bitcast(f32))
    nc.compile()
    return nc


_NC_CACHE = None


def _get_nc():
    global _NC_CACHE
    if _NC_CACHE is None:
        _NC_CACHE = build_nc()
    return _NC_CACHE


def kernel(inp, E, W_r, b_r, W_l, b_l):
    inp = np.asarray(inp).astype(np.int32)
    E = np.ascontiguousarray(np.asarray(E, dtype=np.float32))
    W_r = np.asarray(W_r, dtype=np.float32)
    b_r = np.asarray(b_r, dtype=np.float32)
    W_l = np.asarray(W_l, dtype=np.float32)
    b_l = np.asarray(b_l, dtype=np.float32)

    ident = np.eye(128, dtype=np.float32)
    onesv = np.ones((1, 128), dtype=np.float32)

    in_maps = []
    meta = []
    for core in range(8):
        dirn, bq = divmod(core, 4)
        sl = inp[BL * bq:BL * (bq + 1)]          # [16, 512]
        if dirn:
            sl = sl[:, ::-1]
        tok = sl.T                                # [512, 16] t-major
        idx_tiles = np.ascontiguousarray(
            tok.reshape(NMT, 128).T).astype(np.int32)   # [128, 64]
        W = W_r if dirn == 0 else W_l
        b = b_r if dirn == 0 else b_l
        We = W[:, :EMB]
        Wh = W[:, EMB:]
        bp = b - Wh.sum(axis=1)
        WhT_dev = np.ascontiguousarray(
            Wh.T.reshape(8, 128, HID).transpose(1, 0, 2).reshape(128, 8 * HID))
        WeT_dev = np.ascontiguousarray(
            We.T.reshape(4, 128, HID).transpose(1, 0, 2).reshape(128, 4 * HID))
        in_maps.append({
            "E": E, "idx": idx_tiles, "WhT": WhT_dev, "WeT": WeT_dev,
            "bias": bp.reshape(1, HID).copy(), "ones": onesv, "ident": ident,
            "i16": np.eye(BL, dtype=np.float32),
            "hinit": np.ones((128, 128), dtype=np.float32),
        })
        meta.append((dirn, bq))

    nc = _get_nc()
    kw = {}
    if os.environ.get("BASS_PROFILE") == "1":
        tdir = os.environ.get("BASS_PROFILE_DIR") or tempfile.mkdtemp(
            prefix="bassprof_")
        kw = dict(trace=True, tmpdir=tdir)
        print(f"profiling to {tdir}")
    res = run_bass_kernel_spmd(nc, in_maps, core_ids=list(range(8)), **kw)
    if kw:
        print(f"HW exec time: {res.exec_time_ns} ns")

    out = np.zeros((BATCH, SEQ, 2 * HID), dtype=np.float32)
    for core in range(8):
        dirn, bq = meta[core]
        hsT = res.results[core]["hsT"]            # [512, 128, 128]
        # cols = (c, q, b); j = (4c + q) * 128 + p
        h = hsT.reshape(SEQ, 128, 2, 4, BL)
        h = h.transpose(4, 0, 2, 3, 1).reshape(BL, SEQ, HID) - 1.0
        if dirn:
            h = h[:, ::-1]
        out[BL * bq:BL * (bq + 1), :, HID * dirn:HID * (dirn + 1)] = h
    return out


# revision 19
# speedup vs baseline: 1.0297x; 1.0297x over previous
"""Bidirectional ELU-RNN encoder kernel for Trainium2 (8 NeuronCores).

Sharding: data-parallel over (batch x direction): cores 0-3 run the
left-to-right direction on batch quarters 0-3, cores 4-7 the
right-to-left direction (host reverses time for those cores so every
core runs an identical forward recurrence).

Per core (B=16 sequences, S=512 steps, EMB=512, HID=1024):
  phase 1: indirect-DMA gather of E rows + PE transpose -> embT tiles
  phase 2: X = emb @ We.T + b as a big GEMM (f32r), X -> DRAM
  phase 3: recurrence h_t = elu(X_t + h_{t-1} @ Wh.T)
     - h kept transposed (hT [128, 8kt x 16b]) so it feeds matmul lhsT
     - X_t seeded into PSUM by engine copy, matmuls accumulate on top
     - pre-activation transposed back by PE transposes; ELU computed as
       relu(a) - relu(1 - exp(a)) on full-width [128, *] tiles
"""

import os
import sys
import tempfile

sys.path.insert(0, "/opt/trn_rl_repo")

import numpy as np

from concourse import bass, bacc, mybir
from concourse import hw_specs
from concourse.tile import TileContext
from concourse.bass_utils import run_bass_kernel_spmd

# Prefer the activation-function table that holds BOTH Exp and Ln so the
# per-step Exp/Ln alternation doesn't reload tables (1.28us each).
_orig_get_act_tables = hw_specs.get_activation_tables


def _act_tables_ln_exp_first(module_arch):
    tabs = _orig_get_act_tables(module_arch)
    if "natural_log_exp_and_others" not in tabs:
        return tabs
    exp = mybir.ActivationFunctionType.Exp
    ln = mybir.ActivationFunctionType.Ln
    out = {}
    for name, fns in tabs.items():
        if name != "natural_log_exp_and_others":
            fns = fns - {exp, ln}
        out[name] = fns
    return out


hw_specs.get_activation_tables = _act_tables_ln_exp_first
bacc.get_activation_tables = _act_tables_ln_exp_first

f32 = mybir.dt.float32
f32r = mybir.dt.float32r
i32 = mybir.dt.int32

VOCAB = 32000
EMB = 512
HID = 1024
BATCH = 64
SEQ = 512

BL = 16          # sequences per core
NTOK = SEQ * BL  # 8192 tokens per core
NMT = NTOK // 128  # 64 token m-tiles
SBLK = 8         # steps per X-block DMA
ACT = mybir.ActivationFunctionType


def build_nc():
    nc = bacc.Bacc()

    E = nc.dram_tensor("E", [VOCAB, EMB], f32, kind="ExternalInput")
    idx = nc.dram_tensor("idx", [128, NMT], i32, kind="ExternalInput")
    WhT = nc.dram_tensor("WhT", [128, 8 * HID], f32r, kind="ExternalInput")
    WeT = nc.dram_tensor("WeT", [128, 4 * HID], f32r, kind="ExternalInput")
    bias = nc.dram_tensor("bias", [1, HID], f32r, kind="ExternalInput")
    ones = nc.dram_tensor("ones", [1, 128], f32r, kind="ExternalInput")
    i16 = nc.dram_tensor("i16", [BL, BL], f32r, kind="ExternalInput")
    hin = nc.dram_tensor("hinit", [128, 128], f32r, kind="ExternalInput")
    ident = nc.dram_tensor("ident", [128, 128], f32, kind="ExternalInput")
    hsT = nc.dram_tensor("hsT", [SEQ, 128, 128], f32, kind="ExternalOutput")

    with TileContext(nc) as tc:
        with tc.tile_pool(name="const", bufs=1) as cp, \
             tc.tile_pool(name="gather", bufs=3) as gp, \
             tc.tile_pool(name="xio", bufs=3) as xp, \
             tc.tile_pool(name="work", bufs=6) as wp, \
             tc.tile_pool(name="hbuf", bufs=3) as hp, \
             tc.tile_pool(name="dram", bufs=1, space="DRAM") as dp, \
             tc.tile_pool(name="psbig", bufs=2, space="PSUM") as psb, \
             tc.tile_pool(name="pssm", bufs=4, space="PSUM") as pss:

            # ---- constants / weights ----
            whT = cp.tile([128, 8 * HID], f32r)
            weT = cp.tile([128, 4 * HID], f32r)
            bia = cp.tile([1, HID], f32r)
            one = cp.tile([1, 128], f32r)
            idn = cp.tile([128, 128], f32)
            ixt = cp.tile([128, NMT], i32)
            id16 = cp.tile([BL, BL], f32r)
            nc.sync.dma_start(id16[:], i16[:])
            hinit = cp.tile([128, 128], f32r)
            nc.sync.dma_start(hinit[:], hin[:])
            nc.sync.dma_start(whT[:], WhT[:])
            nc.sync.dma_start(weT[:], WeT[:])
            nc.sync.dma_start(bia[:], bias[:])
            nc.sync.dma_start(one[:], ones[:])
            nc.sync.dma_start(idn[:], ident[:])
            nc.sync.dma_start(ixt[:], idx[:])

            X = dp.tile([NTOK, HID], f32r)

            # ---- phase 1+2: gather + input projection ----
            for m in range(NMT):
                g = gp.tile([128, EMB], f32, tag="g")
                nc.gpsimd.indirect_dma_start(
                    out=g[:], out_offset=None, in_=E[:],
                    in_offset=bass.IndirectOffsetOnAxis(
                        ap=ixt[:, m:m + 1], axis=0))
                embT = gp.tile([128, EMB], f32r, tag="embT")
                for q in range(4):
                    tp = pss.tile([128, 128], f32, tag="trE")
                    nc.tensor.transpose(
                        tp[:], g[:, 128 * q:128 * (q + 1)], idn[:])
                    nc.vector.tensor_copy(
                        embT[:, 128 * q:128 * (q + 1)], tp[:])
                xps = psb.tile([128, HID], f32, tag="big")
                for c in range(2):
                    oc = 512 * c
                    for k in range(4):
                        nc.tensor.matmul(
                            xps[:, oc:oc + 512],
                            lhsT=embT[:, 128 * k:128 * (k + 1)],
                            rhs=weT[:, HID * k + oc:HID * k + oc + 512],
                            start=(k == 0), stop=False)
                    nc.tensor.matmul(
                        xps[:, oc:oc + 512], lhsT=one[:1, :],
                        rhs=bia[:1, oc:oc + 512], start=False, stop=True)
                xsb = gp.tile([128, HID], f32r, tag="xsb")
                nc.vector.tensor_copy(xsb[:], xps[:])
                nc.sync.dma_start(X[128 * m:128 * (m + 1), :], xsb[:])

            # ---- phase 3: recurrence (software-pipelined) ----
            xv = X[:].rearrange("(s b) j -> s b j", b=BL)
            xbs = {}

            def load_xblk(blk):
                xb = xp.tile([BL, SBLK * HID], f32r, tag="xb")
                nc.sync.dma_start(
                    xb[:].rearrange("p (s j) -> p s j", s=SBLK),
                    xv[SBLK * blk:SBLK * (blk + 1), :, :].rearrange(
                        "s b j -> b s j"))
                xbs[blk] = xb

            def seed(t):
                # X_t -> PSUM via a K=16 identity matmul on PE
                blk, s = divmod(t, SBLK)
                if s == 0:
                    load_xblk(blk)
                a = psb.tile([128, HID], f32, tag="big")
                av = a[0:BL, :]
                xb = xbs[blk]
                for c in range(2):
                    oc = 512 * c
                    nc.tensor.matmul(
                        av[:, oc:oc + 512], lhsT=id16[:],
                        rhs=xb[:, HID * s + oc:HID * s + oc + 512],
                        start=True, stop=False, skip_group_check=True)
                return a

            def elu_exp(av, c):
                # Shifted ELU: g = h + 1 = min(exp(a), relu(a) + 1), with
                # relu(a) recovered as max(ln(exp(a)) + 1, 1) so the psum
                # chunk is read exactly once (by ACT exp). The +1 shift is
                # absorbed by bias b' = b - Wh@1 on the host; outputs store
                # g and the host subtracts 1. exp emitted in two halves so
                # the transpose/ln/min tail pipelines behind the first half.
                oc = 512 * c
                ec = wp.tile([BL, 512], f32, tag="ec")
                nc.scalar.activation(ec[:, 0:256], av[:, oc:oc + 256], ACT.Exp)
                nc.scalar.activation(
                    ec[:, 256:512], av[:, oc + 256:oc + 512], ACT.Exp)
                return ec

            def elu_tail(ec, c, h_new):
                for half in range(2):
                    trE = pss.tile([128, 32], f32, tag="trE")
                    for q in range(2):
                        qq = 2 * half + q
                        nc.tensor.transpose(
                            trE[:, 16 * q:16 * (q + 1)],
                            ec[:, 128 * qq:128 * (qq + 1)], idn[0:BL, 0:BL])
                    ln = wp.tile([128, 32], f32, tag="ln")
                    nc.scalar.activation(ln[:], trE[:], ACT.Ln)
                    r1 = wp.tile([128, 32], f32, tag="r1")
                    nc.vector.tensor_scalar(
                        out=r1[:], in0=ln[:], scalar1=1.0, scalar2=1.0,
                        op0=mybir.AluOpType.add, op1=mybir.AluOpType.max)
                    nc.vector.tensor_tensor(
                        out=h_new[:, 64 * c + 32 * half:64 * c + 32 * half + 32],
                        in0=trE[:], in1=r1[:], op=mybir.AluOpType.min)

            def elu_chunk(av, c, h_new):
                elu_tail(elu_exp(av, c), c, h_new)

            def mm_group(av, h_prev, c, klo):
                oc = 512 * c
                for kt in range(klo, klo + 4):
                    nc.tensor.matmul(
                        av[:, oc:oc + 512],
                        lhsT=h_prev[:, 16 * kt:16 * (kt + 1)],
                        rhs=whT[:, HID * kt + oc:HID * kt + oc + 512],
                        start=False, stop=(kt == 7), skip_group_check=True)

            # Software pipeline: chunk-1's post-exp tail (transpose, ln,
            # min) is deferred into the next iteration, emitted after that
            # iteration's first ready mm group, so the in-order PE stream
            # never idles waiting on the exp of the freshest chunk.
            h_prev = hinit
            a_cur = seed(0)
            pend = None  # (ec, h_tile, t) for the deferred c1 tail
            for t in range(SEQ):
                av = a_cur[0:BL, :]
                h_new = hp.tile([128, 128], f32r, tag="hT")
                mm_group(av, h_prev, 0, 0)
                if pend is not None:
                    pec, ph, pt = pend
                    elu_tail(pec, 1, ph)
                    nc.sync.dma_start(hsT[pt], ph[:].bitcast(f32))
                mm_group(av, h_prev, 0, 4)
                mm_group(av, h_prev, 1, 0)
                elu_chunk(av, 0, h_new)
                mm_group(av, h_prev, 1, 4)
                if t < SEQ - 1:
                    a_cur = seed(t + 1)
                ec1 = elu_exp(av, 1)
                pend = (ec1, h_new, t)
                h_prev = h_new
            pec, ph, pt = pend
            elu_tail(pec, 1, ph)
            nc.sync.dma_start(hsT[pt], ph[:].bitcast(f32))
    nc.compile()
    return nc


_NC_CACHE = None


def _get_nc():
    global _NC_CACHE
    if _NC_CACHE is None:
        _NC_CACHE = build_nc()
    return _NC_CACHE


def kernel(inp, E, W_r, b_r, W_l, b_l):
    inp = np.asarray(inp).astype(np.int32)
    E = np.ascontiguousarray(np.asarray(E, dtype=np.float32))
    W_r = np.asarray(W_r, dtype=np.float32)
    b_r = np.asarray(b_r, dtype=np.float32)
    W_l = np.asarray(W_l, dtype=np.float32)
    b_l = np.asarray(b_l, dtype=np.float32)

    ident = np.eye(128, dtype=np.float32)
    onesv = np.ones((1, 128), dtype=np.float32)

    in_maps = []
    meta = []
    for core in range(8):
        dirn, bq = divmod(core, 4)
        sl = inp[BL * bq:BL * (bq + 1)]          # [16, 512]
        if dirn:
            sl = sl[:, ::-1]
        tok = sl.T                                # [512, 16] t-major
        idx_tiles = np.ascontiguousarray(
            tok.reshape(NMT, 128).T).astype(np.int32)   # [128, 64]
        W = W_r if dirn == 0 else W_l
        b = b_r if dirn == 0 else b_l
        We = W[:, :EMB]
        Wh = W[:, EMB:]
        bp = b - Wh.sum(axis=1)
        WhT_dev = np.ascontiguousarray(
            Wh.T.reshape(8, 128, HID).transpose(1, 0, 2).reshape(128, 8 * HID))
        WeT_dev = np.ascontiguousarray(
            We.T.reshape(4, 128, HID).transpose(1, 0, 2).reshape(128, 4 * HID))
        in_maps.append({
            "E": E, "idx": idx_tiles, "WhT": WhT_dev, "WeT": WeT_dev,
            "bias": bp.reshape(1, HID).copy(), "ones": onesv, "ident": ident,
            "i16": np.eye(BL, dtype=np.float32),
            "hinit": np.ones((128, 128), dtype=np.float32),
        })
        meta.append((dirn, bq))

    nc = _get_nc()
    kw = {}
    if os.environ.get("BASS_PROFILE") == "1":
        tdir = os.environ.get("BASS_PROFILE_DIR") or tempfile.mkdtemp(
            prefix="bassprof_")
        kw = dict(trace=True, tmpdir=tdir)
        print(f"profiling to {tdir}")
    res = run_bass_kernel_spmd(nc, in_maps, core_ids=list(range(8)), **kw)
    if kw:
        print(f"HW exec time: {res.exec_time_ns} ns")

    out = np.zeros((BATCH, SEQ, 2 * HID), dtype=np.float32)
    for core in range(8):
        dirn, bq = meta[core]
        hsT = res.results[core]["hsT"]            # [512, 128, 128]
        # cols = (c, q, b); j = (4c + q) * 128 + p
        h = hsT.reshape(SEQ, 128, 2, 4, BL)
        h = h.transpose(4, 0, 2, 3, 1).reshape(BL, SEQ, HID) - 1.0
        if dirn:
            h = h[:, ::-1]
        out[BL * bq:BL * (bq + 1), :, HID * dirn:HID * (dirn + 1)] = h
    return out
